# revision 22
# baseline (speedup 1.0000x reference)
"""Trainium2 Bass kernel: 16-head attention (S=1024, hidden=1024) + output
linear, data-parallel over the batch dimension (8 batch elements -> 8 cores).

Contract: kernel(**inputs) takes the FULL unsharded inputs of
nn_Attention_83915071029891 and returns the FULL (8, 1024, 1024) f32 output.

Per-core algorithm (transposed-scores layout; TensorE contracts over the
partition dim, so scores are built k-major and never transposed):
  per head-pair (head A on partitions 0-63, head B on 64-127), per 512-wide
  q-pass, per k-tile:
    scoresT = kT-stationary.T @ qT                 (row-packed A||B matmuls)
    exp split between two engines at k-tile granularity (NA of 8 k-tiles
    go to the DVE):
      ScalarE ACT exp on most k-tiles                        (exact)
      DVE two-term Schraudolph on the remaining k-tiles: int16 codes
        i=round(s*L+B) and i-65; bitcast to bf16; the two terms are
        summed implicitly by accumulating both AV/den matmuls into PSUM
    AV col-tiled: vA stationary @ PE cols 0-63, vB @ cols 64-127, so head B
      lands directly on PSUM partitions 64-127 (no cross-partition staging)
    den col-tiled: ones-stationary M=1 matmuls @ PE cols 0 and 32
  per pair-pass: reciprocal of den rows, DMA-broadcast to a [128,512] tile,
    one fused evacuate+normalize tensor_mul into outT
  fc: y = outT-stationary.T @ fc_wT + fc_b, interleaved as half-qtile chunks
    between the second q-pass's pairs to overlap with attention
"""

import os
import sys

for _p in ("/opt/trn_rl_repo", "/root/.axon_site/_ro/trn_rl_repo"):
    if _p not in sys.path:
        sys.path.append(_p)

from contextlib import ExitStack

import numpy as np

import bass_rust
import concourse.bass as bass
import concourse.mybir as mybir
import concourse.tile as tile
from concourse.vector_clock import ScopedClock

F32 = mybir.dt.float32
BF16 = mybir.dt.bfloat16
I16 = mybir.dt.int16
AF = mybir.ActivationFunctionType
ALU = mybir.AluOpType

N_CORES = 8
_MAX_CTRL_WAITS = 1

# two-term Schraudolph exp constants (bf16 codes): exp(s/32) ~
#   bitcast(round(s*LP + B1)) + bitcast(round(s*LP + B1) - 65)
# B1 is offset by -128*log2(1+2^(-65/128)) so the two-term SUM has unit mean
# ratio vs exp (it mixes with exact-exp k-tiles inside one softmax row), then
# tuned numerically for zero mean and min RMS deviation.
LP = float(128.0 / np.log(2.0) / 32.0)
B1 = 16150.30
SHIFT = -65.0


def _patched_drain_and_barrier(self, tick_clock, wait_clock):
    """Tile's kernel-tail Drain aggregates one sem wait per outstanding proc,
    but walrus CoreV3 codegen only has one sync-wait slot on CTRL ops -- split
    the waits across a chain of SP drain instructions."""
    nc = self.nc
    drain_inst = nc.sync.drain()
    wait_clock.add_sem_waits(
        drain_inst.ins, ScopedClock({None: tick_clock.global_clock})
    )
    si = drain_inst.ins.sync_info
    if si is not None and si.on_wait and len(si.on_wait) > _MAX_CTRL_WAITS:
        waits = list(si.on_wait)
        drain_inst.ins.sync_info = bass_rust.SyncInfo(
            on_wait=waits[:_MAX_CTRL_WAITS], on_update=list(si.on_update or [])
        )
        for i in range(_MAX_CTRL_WAITS, len(waits), _MAX_CTRL_WAITS):
            extra = nc.sync.drain()
            extra.ins.sync_info = bass_rust.SyncInfo(
                on_wait=waits[i : i + _MAX_CTRL_WAITS], on_update=[]
            )

    nc.all_engine_barrier()
    assert self.sems is not None
    popped = nc._tile_sem_poison_stack.pop()
    assert popped is self._sem_poison
    nc.clear_and_free_semaphores(list(self.sems.allocated().values()))
    nc.all_engine_barrier()


tile.TileContext._drain_and_barrier = _patched_drain_and_barrier


def _split_excess_waits(nc, max_waits=_MAX_CTRL_WAITS):
    """walrus CoreV3 setupSyncWait only has one sync-wait slot per
    instruction; hoist excess sem waits onto same-engine NoOp carriers
    inserted immediately before the over-limit instruction."""
    ctr = [0]

    def carrier(engine, waits):
        ctr[0] += 1
        nop = mybir.InstNoOp(name=f"I-waitc-{ctr[0]}", ins=[], outs=[])
        nop.engine = engine
        nop.sync_info = bass_rust.SyncInfo(on_wait=waits, on_update=[])
        return nop

    for fn in nc.m.functions:
        for blk in fn.blocks:
            il = blk.instructions
            newl = []
            changed = False
            for inst in il:
                si = inst.sync_info
                nw = len(si.on_wait) if si and si.on_wait else 0
                if nw > max_waits:
                    waits = list(si.on_wait)
                    for i in range(max_waits, len(waits), max_waits):
                        newl.append(carrier(inst.engine, waits[i : i + max_waits]))
                    inst.sync_info = bass_rust.SyncInfo(
                        on_wait=waits[:max_waits], on_update=list(si.on_update or [])
                    )
                    changed = True
                newl.append(inst)
            if changed:
                il.clear()
                il.extend(newl)
                assert len(blk.instructions) == len(newl), (
                    "block instruction list is not a live reference"
                )


def build_kernel(S=1024, HEADS=16, NA=2, split_waits=True):
    """Trace the per-core Bass program. DRAM io: qT,kT,v,fc_wT,fc_b -> y.

    NA: number of k-tiles (of 8) per pair-pass whose exp is computed by the
    DVE two-term Schraudolph instead of ScalarE ACT. NA=0 -> pure ScalarE.
    """
    HD = 64
    H = HEADS * HD
    KT = S // 128
    PAIRS = HEADS // 2
    ITILES = H // 128
    QW = 512                       # q-pass width
    NQP = S // QW                  # q-passes
    # approx ktiles spread through the pair so ScalarE/DVE pipeline
    approx_t = set((3, 7, 5, 1, 6, 2, 4, 0)[:NA])
    SCALE = 1.0 / float(H) ** 0.5

    nc = bass.Bass(trn_type="TRN2")

    qT = nc.dram_tensor("qT", [H, S], BF16, kind="ExternalInput").ap()
    kT = nc.dram_tensor("kT", [H, S], BF16, kind="ExternalInput").ap()
    vt = nc.dram_tensor("vt", [HEADS, 128, KT * HD], BF16, kind="ExternalInput").ap()
    fc_wT = nc.dram_tensor("fc_wT", [H, H], BF16, kind="ExternalInput").ap()
    fc_b = nc.dram_tensor("fc_b", [1, H], F32, kind="ExternalInput").ap()
    ones128 = nc.dram_tensor("ones128", [128, 1], BF16, kind="ExternalInput").ap()
    y = nc.dram_tensor("y", [S, H], F32, kind="ExternalOutput").ap()

    with tile.TileContext(nc) as tc:
        with ExitStack() as ctx:
            big = ctx.enter_context(tc.tile_pool(name="big", bufs=1))
            at = ctx.enter_context(tc.tile_pool(name="at", bufs=3))
            it = ctx.enter_context(tc.tile_pool(name="it", bufs=4))
            tl = ctx.enter_context(tc.tile_pool(name="tl", bufs=3))
            rp = ctx.enter_context(tc.tile_pool(name="rp", bufs=2))
            yp = ctx.enter_context(tc.tile_pool(name="yp", bufs=2))
            # PSUM (8 banks): ps 2x[128,1024]=4, po 2x[128,512]=2,
            # pd 2x([33,512] den | [128,512] fc accum)=2
            ps = ctx.enter_context(tc.tile_pool(name="ps", bufs=2, space="PSUM"))
            po = ctx.enter_context(tc.tile_pool(name="po", bufs=2, space="PSUM"))
            pd = ctx.enter_context(tc.tile_pool(name="pd", bufs=2, space="PSUM"))

            on128 = big.tile([128, 1], BF16, tag="on128")
            nc.sync.dma_start(out=on128[:, :], in_=ones128[:, :])

            # resident operands; pair-ordered loads so pair 0 starts early
            kT_sb = big.tile([128, PAIRS * S], BF16, tag="kT")
            qT_sb = big.tile([128, PAIRS * S], BF16, tag="qT")
            v_sb = big.tile([128, HEADS * KT * HD], BF16, tag="v")
            for p in range(PAIRS):
                psl = slice(128 * p, 128 * (p + 1))
                csl = slice(S * p, S * (p + 1))
                nc.sync.dma_start(out=kT_sb[:, csl], in_=kT[psl, :])
                nc.sync.dma_start(out=qT_sb[:, csl], in_=qT[psl, :])
                for h in (2 * p, 2 * p + 1):
                    nc.sync.dma_start(
                        out=v_sb[:, KT * HD * h : KT * HD * (h + 1)], in_=vt[h]
                    )
            # fc weights ride the (otherwise idle) GPSIMD SWDGE ring so they
            # never queue ahead of the attention input loads
            fcw_sb = big.tile([128, ITILES * H], BF16, tag="fcw")
            fcb_sb = big.tile([128, H], F32, tag="fcb")
            for i in range(ITILES):
                nc.gpsimd.dma_start(
                    out=fcw_sb[:, H * i : H * (i + 1)],
                    in_=fc_wT[128 * i : 128 * (i + 1), :],
                )
            nc.gpsimd.dma_start(
                out=fcb_sb[:, :], in_=fc_b.unsqueeze(1).broadcast_to((1, 128, H))
            )

            outT_sb = big.tile([128, ITILES * S], BF16, tag="outT")

            # PE issue order is strict FIFO per engine: software-pipeline so
            # each ktile's AV/den (which wait on that ktile's exp) are issued
            # AFTER the next ktile's QK, keeping PE busy during the exp.
            pend_avden = [None]
            # previous pair's normalization path, staged across the next
            # pair's ktiles so the in-order DVE queue never stalls on the
            # DMA links of the den->reciprocal->broadcast chain
            pend_stages = []

            def flush_pending():
                if pend_avden[0] is not None:
                    pend_avden[0]()
                    pend_avden[0] = None

            def flush_stages(t):
                while pend_stages and pend_stages[0][0] <= t:
                    pend_stages.pop(0)[1]()

            def make_avden(poAB, den, vA, vB, aAB, i16, j16, t):
                vsl = slice(HD * t, HD * (t + 1))
                st, sp_ = (t == 0), (t == KT - 1)

                def emit_quad(rhsA, rhsB, start, stop):
                    nc.tensor.matmul(
                        poAB[0:64, :], vA[:, vsl], rhsA,
                        start=start, stop=stop, tile_position=(0, 0),
                    )
                    nc.tensor.matmul(
                        poAB[64:128, :], vB[:, vsl], rhsB,
                        start=start, stop=stop, tile_position=(0, 64),
                    )
                    nc.tensor.matmul(
                        den[0:1, :], on128[:, :], rhsA,
                        start=start, stop=stop, tile_position=(0, 0),
                    )
                    nc.tensor.matmul(
                        den[32:33, :], on128[:, :], rhsB,
                        start=start, stop=stop, tile_position=(0, 32),
                    )

                def emit():
                    if aAB is not None:
                        emit_quad(aAB[:, 0:QW], aAB[:, QW : 2 * QW], st, sp_)
                    else:
                        # two code-term streams accumulate the two-term exp
                        # sum directly in PSUM
                        for term, code in ((0, i16), (1, j16)):
                            emit_quad(
                                code[:, 0:QW].bitcast(BF16),
                                code[:, QW : 2 * QW].bitcast(BF16),
                                st and term == 0, sp_ and term == 1,
                            )

                return emit

            def make_finish(poAB, den, p, q0):
                box = {}

                def stage1():
                    # evacuate the two denominator rows (PSUM is not DMA-
                    # readable; lanes run in parallel so one copy is cheap)
                    denS = tl.tile([33, QW], F32, tag="denS")
                    nc.vector.tensor_copy(denS[:, :], den[0:33, :])
                    denP = tl.tile([128, 8], F32, tag="denP")
                    nc.sync.dma_start(
                        out=denP[:, 0:4],
                        in_=denS[0:1, :].rearrange("p (a b) -> p a b", b=4),
                    )
                    nc.sync.dma_start(
                        out=denP[:, 4:8],
                        in_=denS[32:33, :].rearrange("p (a b) -> p a b", b=4),
                    )
                    box["denP"] = denP

                def stage2():
                    denP = box["denP"]
                    recP = tl.tile([128, 8], F32, tag="recP")
                    nc.vector.reciprocal(recP[:, :], denP[:, :])
                    rec = tl.tile([1, 2 * QW], F32, tag="rec")
                    nc.sync.dma_start(
                        out=rec[:, 0:QW].rearrange("p (a b) -> p a b", b=4),
                        in_=recP[:, 0:4],
                    )
                    nc.sync.dma_start(
                        out=rec[:, QW : 2 * QW].rearrange("p (a b) -> p a b", b=4),
                        in_=recP[:, 4:8],
                    )
                    # broadcast reciprocals across partitions via DMA (rows
                    # 0-63 <- recA, 64-127 <- recB)
                    RpS = rp.tile([128, QW], F32, tag="RpS")
                    nc.sync.dma_start(
                        out=RpS[0:64, :],
                        in_=rec[:, 0:QW].unsqueeze(1).broadcast_to((1, 64, QW)),
                    )
                    nc.sync.dma_start(
                        out=RpS[64:128, :],
                        in_=rec[:, QW : 2 * QW].unsqueeze(1).broadcast_to((1, 64, QW)),
                    )
                    box["RpS"] = RpS

                def stage3():
                    # fused evacuate + normalize: outT tile for pair p
                    nc.vector.tensor_mul(
                        outT_sb[:, S * p + q0 : S * p + q0 + QW],
                        poAB[:, :], box["RpS"][:, :],
                    )

                return [(0, stage1), (2, stage2), (5, stage3)]

            fc_pending = []

            for qp in range(NQP):
                q0 = QW * qp
                qsl = slice(q0, q0 + QW)
                for p in range(PAIRS):
                    kTp = kT_sb[:, S * p : S * (p + 1)]
                    qTp = qT_sb[:, S * p : S * (p + 1)]
                    vA = v_sb[:, KT * HD * 2 * p : KT * HD * (2 * p + 1)]
                    vB = v_sb[:, KT * HD * (2 * p + 1) : KT * HD * (2 * p + 2)]

                    poAB = po.tile([128, QW], F32, tag="po")
                    den = pd.tile([33, QW], F32, tag="pd")

                    # one fc half-qtile chunk rides along with this pair,
                    # one matmul per ktile, filling PE exp-wait bubbles
                    pyh = None
                    if fc_pending:
                        fqt, fo0 = fc_pending.pop(0)
                        pyh = pd.tile([128, QW], F32, tag="pd")

                    for t in range(KT):
                        ksl = slice(128 * t, 128 * (t + 1))
                        sAB = ps.tile([128, 2 * QW], F32, tag="s")
                        mmA = nc.tensor.matmul(
                            sAB[:, 0:QW], kTp[0:64, ksl], qTp[0:64, qsl],
                            start=True, stop=True,
                        )
                        mmB = nc.tensor.matmul(
                            sAB[:, QW : 2 * QW], kTp[64:128, ksl], qTp[64:128, qsl],
                            start=True, stop=True,
                        )
                        # keep the K=64 row-group pair adjacent on PE so the
                        # two half-array matmuls run concurrently
                        tile.add_dep_helper(mmB.ins, mmA.ins, sync=False, reason="rowpack")

                        aAB = i16 = j16 = None
                        if t in approx_t:
                            i16 = it.tile([128, 2 * QW], I16, tag="i16")
                            nc.vector.tensor_scalar(
                                i16[:, :], sAB[:, :], LP, B1, ALU.mult, ALU.add
                            )
                            j16 = it.tile([128, 2 * QW], I16, tag="j16")
                            nc.vector.tensor_scalar(
                                j16[:, :], i16[:, :], SHIFT, None, ALU.add
                            )
                        else:
                            aAB = at.tile([128, 2 * QW], BF16, tag="a")
                            nc.scalar.activation(
                                aAB[:, :], sAB[:, :], AF.Exp, scale=SCALE
                            )

                        flush_pending()
                        flush_stages(t)
                        if pyh is not None:
                            nc.tensor.matmul(
                                pyh[:, :],
                                outT_sb[:, S * t + 128 * fqt : S * t + 128 * (fqt + 1)],
                                fcw_sb[:, H * t + fo0 : H * t + fo0 + QW],
                                start=(t == 0), stop=(t == KT - 1),
                            )
                        pend_avden[0] = make_avden(poAB, den, vA, vB, aAB, i16, j16, t)

                    if pyh is not None:
                        ysb = yp.tile([128, QW], F32, tag="ysb")
                        nc.vector.tensor_add(
                            ysb[:, :], pyh[:, :], fcb_sb[:, fo0 : fo0 + QW]
                        )
                        nc.sync.dma_start(
                            out=y[128 * fqt : 128 * (fqt + 1), fo0 : fo0 + QW],
                            in_=ysb[:, :],
                        )
                    pend_stages.extend(make_finish(poAB, den, p, q0))

                # queue this q-pass's fc work (4 qtiles x 2 o-halves)
                for qt in range(q0 // 128, (q0 + QW) // 128):
                    for o0 in (0, QW):
                        fc_pending.append((qt, o0))

            flush_pending()
            flush_stages(KT)
            # trailing fc chunks (the last q-pass's work)
            while fc_pending:
                fqt, fo0 = fc_pending.pop(0)
                pyh = pd.tile([128, QW], F32, tag="pd")
                for i in range(ITILES):
                    nc.tensor.matmul(
                        pyh[:, :],
                        outT_sb[:, S * i + 128 * fqt : S * i + 128 * (fqt + 1)],
                        fcw_sb[:, H * i + fo0 : H * i + fo0 + QW],
                        start=(i == 0), stop=(i == ITILES - 1),
                    )
                ysb = yp.tile([128, QW], F32, tag="ysb")
                nc.vector.tensor_add(ysb[:, :], pyh[:, :], fcb_sb[:, fo0 : fo0 + QW])
                nc.sync.dma_start(
                    out=y[128 * fqt : 128 * (fqt + 1), fo0 : fo0 + QW], in_=ysb[:, :]
                )

    if split_waits:
        _split_excess_waits(nc)
    return nc


MM_DTYPE = "bf16"
NA = int(os.environ.get("KNA", "2"))

_CACHED_NC = None


def _get_nc():
    global _CACHED_NC
    if _CACHED_NC is None:
        _CACHED_NC = build_kernel(NA=NA)
    return _CACHED_NC


def prep_core_inputs(q_n, k_n, v_n, fc_wT, fc_b1, HEADS=16):
    """Host-side layout prep for one batch element."""
    import ml_dtypes

    bf = ml_dtypes.bfloat16
    HD = 64
    S, H = q_n.shape
    KT = S // 128
    qT = np.ascontiguousarray(q_n.T).astype(bf)
    kT = np.ascontiguousarray(k_n.T).astype(bf)
    v4 = v_n.reshape(KT, 128, HEADS, HD)  # [t, p, h, d]
    vt = np.ascontiguousarray(v4.transpose(2, 1, 0, 3).reshape(HEADS, 128, KT * HD))
    return {
        "qT": qT,
        "kT": kT,
        "vt": vt.astype(bf),
        "fc_wT": fc_wT.astype(bf),
        "fc_b": fc_b1,
        "ones128": np.ones((128, 1), dtype=bf),
    }


def make_in_maps(key, value, query, fc_w, fc_b):
    key = np.asarray(key, dtype=np.float32)
    value = np.asarray(value, dtype=np.float32)
    query = np.asarray(query, dtype=np.float32)
    fc_w = np.asarray(fc_w, dtype=np.float32)
    fc_b = np.asarray(fc_b, dtype=np.float32)
    N, S, H = query.shape
    fc_wT = np.ascontiguousarray(fc_w.T)
    fc_b1 = np.ascontiguousarray(fc_b.reshape(1, H))
    return [
        prep_core_inputs(query[n], key[n], value[n], fc_wT, fc_b1)
        for n in range(N)
    ]


def run_on_device(in_maps):
    from concourse.bass_utils import run_bass_kernel_spmd

    nc = _get_nc()
    res = run_bass_kernel_spmd(nc, in_maps, list(range(N_CORES)))
    return np.stack([res.results[i]["y"] for i in range(N_CORES)], axis=0)


def kernel(key, value, query, fc_w, fc_b):
    """Full inputs in, full output out. Shards batch N=8 across 8 cores."""
    in_maps = make_in_maps(key, value, query, fc_w, fc_b)
    return run_on_device(in_maps)
